# revision 59
# baseline (speedup 1.0000x reference)
"""DeltaNet Bass kernel for Trainium2, 8-core SPMD — fp16 matmul pipeline.

Sharding: core = (b, h) for b in 0..1, h in 0..3  (b*4 + h).
Each core computes the full per-(batch,head) pipeline and its partial
output projection out_partial[L, D] in fp16; the host sums the 4
head-partials per batch.

Device pipeline per core (PSUM accumulate fp32, operands fp16):
  phase A (per 512-col tile, all 8 tiles first): q/k/v/small projections
          (PE, fp16), causal 4-tap conv (DVE STT chain, fp16 4x mode),
          SiLU evict (ACT) into persistent SBUF: rollkq (chan-major k|q
          interleaved per 128-token chunk) and vwin (guarded chan-major v).
  phase B prologue: beta + gate columns from rows scratch (one Sigmoid /
          Exp table load each).
  phase B (per 128-token chunk): PE transposes (fp16 PSUM), l2norm stats,
          UT-transform T^T via Neumann product, chunk-local prepass,
          serial scan (u = u0 - w S, o = q S + attn u, S += k^T u) with
          fp32 S master + fp16 S copy; FIR-long diag matmuls + FIR-short
          STT chains interleaved per tile to fill PE gaps.
  phase C (per chunk, pipelined one tile behind): FIR transposes,
          hierarchical gating (DVE fp16), RMSNorm, output projection.
"""
import numpy as np
from contextlib import ExitStack

import concourse.bass as bass
import concourse.tile as tile
from concourse import bacc, mybir
from concourse.bass_utils import run_bass_kernel_spmd

F32 = mybir.dt.float32
F16 = mybir.dt.float16
AF = mybir.ActivationFunctionType
ALU = mybir.AluOpType

B, D, H, DK, DV = 2, 1024, 4, 256, 256
CONV_K, FIR_S, FIR_L = 4, 3, 31
CH = 128          # scan chunk (token tile)
NTILE = 512       # column tile for projections / FIR
P = 128
KT = D // P       # 8 contraction tiles over D
GUARD = CONV_K - 1
EPS_RMS = 1e-5
FGUARD = FIR_L    # guard cols ahead of token 0 in vwin


def build(L=4096):
    NT = L // NTILE
    NCH = L // CH
    CPN = NTILE // CH  # chunks per n-tile (4)

    nc = bacc.Bacc("TRN2", target_bir_lowering=False, debug=False, num_devices=8)

    xT_d = nc.dram_tensor("xT", [D, L], F16, kind="ExternalInput").ap()
    wq_d = nc.dram_tensor("wq", [D, DK], F16, kind="ExternalInput").ap()
    wk_d = nc.dram_tensor("wk", [D, DK], F16, kind="ExternalInput").ap()
    wv_d = nc.dram_tensor("wv", [D, DV], F16, kind="ExternalInput").ap()
    wsm_d = nc.dram_tensor("wsm", [D, 5], F16, kind="ExternalInput").ap()
    bias5_d = nc.dram_tensor("bias5", [5], F32, kind="ExternalInput").ap()
    # conv taps per (tensor, pt): [3, 2, 128, 4]
    ctaps_d = nc.dram_tensor("ctaps", [3, 2, P, CONV_K], F32, kind="ExternalInput").ap()
    # fir long-residual diagonal matrices: [pt=2, 31, 128, 128]
    fdiag_d = nc.dram_tensor("fdiag", [2, FIR_L, P, P], F16, kind="ExternalInput").ap()
    # fir short-residual taps: [2, 128, 3]
    staps_d = nc.dram_tensor("staps", [2, P, FIR_S], F32, kind="ExternalInput").ap()
    wo_d = nc.dram_tensor("wo", [DV, D], F16, kind="ExternalInput").ap()
    ident_d = nc.dram_tensor("ident", [P, P], F16, kind="ExternalInput").ap()
    masklt_d = nc.dram_tensor("masklt", [P, P], F16, kind="ExternalInput").ap()  # strict lower
    maskut_d = nc.dram_tensor("maskut", [P, P], F16, kind="ExternalInput").ap()  # upper incl diag
    out_d = nc.dram_tensor("out", [L, D], F16, kind="ExternalOutput").ap()

    with tile.TileContext(nc) as tc, ExitStack() as ctx:
        # ---------------- pools ----------------
        const = ctx.enter_context(tc.tile_pool(name="const", bufs=1))
        bigw = ctx.enter_context(tc.tile_pool(name="bigw", bufs=1))
        persist = ctx.enter_context(tc.tile_pool(name="persist", bufs=1))
        xtp = ctx.enter_context(tc.tile_pool(name="xtp", bufs=2))
        prep = ctx.enter_context(tc.tile_pool(name="prep", bufs=1))
        colp = ctx.enter_context(tc.tile_pool(name="colp", bufs=1))
        chk = ctx.enter_context(tc.tile_pool(name="chk", bufs=1))
        sp = ctx.enter_context(tc.tile_pool(name="sp", bufs=1))
        gat = ctx.enter_context(tc.tile_pool(name="gat", bufs=1))
        dram = ctx.enter_context(tc.tile_pool(name="dram", bufs=1, space="DRAM"))
        ps_big = ctx.enter_context(tc.tile_pool(name="ps_big", bufs=2, space="PSUM"))
        ps_med = ctx.enter_context(tc.tile_pool(name="ps_med", bufs=3, space="PSUM"))
        ps_t = ctx.enter_context(tc.tile_pool(name="ps_t", bufs=2, space="PSUM"))
        ps_fir = ctx.enter_context(tc.tile_pool(name="ps_fir", bufs=1, space="PSUM"))



        # ---------------- constants / weights ----------------
        wq = bigw.tile([P, KT, DK], F16, tag="wq")
        nc.sync.dma_start(wq[:], wq_d.rearrange("(kt p) m -> p kt m", p=P))
        wk = bigw.tile([P, KT, DK], F16, tag="wk")
        nc.sync.dma_start(wk[:], wk_d.rearrange("(kt p) m -> p kt m", p=P))
        wv = bigw.tile([P, KT, DV], F16, tag="wv")
        nc.sync.dma_start(wv[:], wv_d.rearrange("(kt p) m -> p kt m", p=P))
        wsm = const.tile([P, KT, 5], F16)
        nc.sync.dma_start(wsm[:], wsm_d.rearrange("(kt p) m -> p kt m", p=P))
        ident = const.tile([P, P], F16)
        nc.sync.dma_start(ident[:], ident_d)
        masklt = const.tile([P, P], F16)
        nc.sync.dma_start(masklt[:], masklt_d)
        maskut = const.tile([P, P], F16)
        nc.sync.dma_start(maskut[:], maskut_d)
        bias5 = const.tile([5, 1], F32)
        nc.sync.dma_start(bias5[:], bias5_d.rearrange("(m o) -> m o", o=1))
        ctaps = const.tile([P, 3, 2, CONV_K], F32, name="ctaps")
        nc.sync.dma_start(ctaps[:], ctaps_d.rearrange("t pt p j -> p t pt j"))
        staps = const.tile([P, 2, FIR_S], F32, name="staps")
        nc.sync.dma_start(staps[:], staps_d.rearrange("pt p j -> p pt j"))
        fdiag = bigw.tile([P, 2, FIR_L, P], F16, tag="fdiag", name="fdiag")
        nc.sync.dma_start(fdiag[:], fdiag_d.rearrange("pt j p q -> p pt j q"))
        wo = bigw.tile([P, 2, D], F16, tag="wo", name="wo")
        nc.sync.dma_start(wo[:], wo_d.rearrange("(kt p) m -> p kt m", p=P))
        zeros3 = const.tile([P, GUARD], F16)
        nc.vector.memset(zeros3[:], 0.0)
        eps_l2 = const.tile([P, 1], F32)
        nc.vector.memset(eps_l2[:], 1e-6)
        eps_rms = const.tile([P, 1], F32)
        nc.vector.memset(eps_rms[:], EPS_RMS)

        # ---------------- persistent SBUF state ----------------
        # gate columns: [P, chunk, (beta_raw, wg_raw, l0, l1, l2, pad*3)] token-major
        gcols = persist.tile([P, NCH, 8], F16, name="gcols")
        # chan-major post-silu k|q interleaved per chunk: [P, pt, chunk, (k|q), CH]
        rollkq = persist.tile([P, 2, NCH, 2, CH], F16, name="rollkq")
        # chan-major post-silu v with FGUARD leading guard cols
        vwin = persist.tile([P, 2, FGUARD + L], F16, name="vwin")
        nc.vector.memset(vwin[:, :, 0:FGUARD], 0.0)
        # per-chunk outputs of the scan + beta-scaled v (token-major)
        o_all = persist.tile([P, NCH, DV], F16, name="o_all")
        vb_all = persist.tile([P, NCH, DV], F16, name="vb_all")

        S16 = [sp.tile([P, 2, DV], F16, name=f"S16_{i}") for i in range(2)]

        TENS = ("q", "k", "v")
        W_OF = {"q": wq, "k": wk, "v": wv}
        prev_pre = {}

        # ================= phase A: projections + conv + silu =================
        def phaseA(n):
            xt = xtp.tile([P, KT, NTILE], F16, tag="xt", name="xt")
            nc.sync.dma_start(xt[:], xT_d[:, n * NTILE:(n + 1) * NTILE]
                              .rearrange("(kt p) t -> p kt t", p=P))
            for t in TENS:
                for pt in range(2):
                    ps = ps_big.tile([P, NTILE], F32, tag="psb", name=f"ps_{t}{pt}")
                    for kt in range(KT):
                        nc.tensor.matmul(ps[:], W_OF[t][:, kt, pt * P:(pt + 1) * P],
                                         xt[:, kt, :], start=(kt == 0), stop=(kt == KT - 1))
                    key = f"pre{t}{pt}"
                    pre = prep.tile([P, GUARD + NTILE], F16, tag=key, bufs=2, name=key)
                    if n == 0:
                        nc.scalar.copy(pre[:, 0:GUARD], zeros3[:])
                    else:
                        nc.scalar.copy(pre[:, 0:GUARD], prev_pre[key][:, NTILE:NTILE + GUARD])
                    nc.scalar.copy(pre[:, GUARD:], ps[:])
                    prev_pre[key] = pre
                    # conv: 4-tap chain. q/k on DVE in TS+TT form (tensor_scalar
                    # and tensor_tensor hit the 2x 16-bit DVE mode; STT does
                    # not). v on the idle Pool engine (TT-only, broadcast taps).
                    ti = TENS.index(t)
                    acc = prep.tile([P, NTILE], F16, tag="cacc", bufs=2, name="cacc")
                    if t == "v" and pt == 1:
                        bct = lambda j: ctaps[:, ti, pt, j:j + 1].to_broadcast((P, NTILE))
                        nc.gpsimd.tensor_tensor(acc[:], pre[:, 0:NTILE], bct(0), op=ALU.mult)
                        for j in range(1, CONV_K):
                            prod = prep.tile([P, NTILE], F16, tag="cprod", bufs=2, name="cprod")
                            nc.gpsimd.tensor_tensor(prod[:], pre[:, j:j + NTILE], bct(j),
                                                    op=ALU.mult)
                            nc.gpsimd.tensor_tensor(acc[:], acc[:], prod[:], op=ALU.add)
                    else:
                        nc.vector.tensor_scalar_mul(acc[:], pre[:, 0:NTILE],
                                                    ctaps[:, ti, pt, 0:1])
                        for j in range(1, CONV_K):
                            prod = prep.tile([P, NTILE], F16, tag="cprod", bufs=2, name="cprod")
                            nc.vector.tensor_scalar_mul(prod[:], pre[:, j:j + NTILE],
                                                        ctaps[:, ti, pt, j:j + 1])
                            nc.vector.tensor_tensor(acc[:], acc[:], prod[:], op=ALU.add)
                    # silu -> persistent layout
                    if t == "v":
                        nc.scalar.activation(
                            vwin[:, pt, FGUARD + n * NTILE: FGUARD + (n + 1) * NTILE],
                            acc[:], AF.Silu)
                    else:
                        koff = 0 if t == "k" else 1
                        nc.scalar.activation(
                            rollkq[:, pt, n * CPN:(n + 1) * CPN, koff, :],
                            acc[:], AF.Silu)
            # small projections [5, NTILE]
            ps5 = ps_big.tile([P, NTILE], F32, tag="psb", name="ps5")
            for kt in range(KT):
                nc.tensor.matmul(ps5[:5, :], wsm[:, kt, :], xt[:, kt, :],
                                 start=(kt == 0), stop=(kt == KT - 1))
            rows_sb = colp.tile([5, NTILE], F16, tag="rows_sb", bufs=2, name="rows_sb")
            nc.scalar.activation(rows_sb[:], ps5[:5, :], AF.Identity, bias=bias5[:])
            # token-major gate columns via tiny PE transposes (no DRAM round trip)
            rtp = ps_t.tile([P, 2, CH], F16, tag="pst", name="rtp")
            for ci in range(CPN):
                nc.tensor.transpose(rtp[:, 0, ci * 8:ci * 8 + 5],
                                    rows_sb[:, ci * CH:(ci + 1) * CH], ident[:5, :5])
            nc.scalar.copy(gcols[:, n * CPN:(n + 1) * CPN, :], rtp[:, 0, 0:8 * CPN])

        for n in range(NT):
            phaseA(n)

        # ================= phase B prologue: beta + gate columns =================
        beta_all = colp.tile([P, NCH], F32, tag="beta_all", bufs=1, name="beta_all")
        nc.scalar.activation(beta_all[:], gcols[:, :, 0], AF.Sigmoid)
        wg_c = colp.tile([P, NCH], F32, tag="wg_c", bufs=1, name="wg_c")
        nc.scalar.activation(wg_c[:], gcols[:, :, 1], AF.Sigmoid)
        mx = colp.tile([P, NCH], F32, tag="mx", bufs=1, name="mx")
        nc.vector.tensor_tensor(mx[:], gcols[:, :, 2], gcols[:, :, 3], op=ALU.max)
        nc.vector.tensor_tensor(mx[:], mx[:], gcols[:, :, 4], op=ALU.max)
        e0 = colp.tile([P, NCH], F32, tag="e0", bufs=1, name="e0")
        e1 = colp.tile([P, NCH], F32, tag="e1", bufs=1, name="e1")
        e2 = colp.tile([P, NCH], F32, tag="e2", bufs=1, name="e2")
        for r, dst in ((2, e0), (3, e1), (4, e2)):
            nc.vector.tensor_tensor(dst[:], gcols[:, :, r], mx[:], op=ALU.subtract)
            nc.scalar.activation(dst[:], dst[:], AF.Exp)
        esum = colp.tile([P, NCH], F32, tag="esum", bufs=1, name="esum")
        nc.vector.tensor_tensor(esum[:], e0[:], e1[:], op=ALU.add)
        nc.vector.tensor_tensor(esum[:], esum[:], e2[:], op=ALU.add)
        erec = colp.tile([P, NCH], F32, tag="erec", bufs=1, name="erec")
        nc.vector.reciprocal(erec[:], esum[:])
        p1 = colp.tile([P, NCH], F32, tag="p1", bufs=1, name="p1")
        p2 = colp.tile([P, NCH], F32, tag="p2", bufs=1, name="p2")
        for src, dst in ((e1, p1), (e2, p2)):
            nc.vector.tensor_tensor(dst[:], src[:], erec[:], op=ALU.mult)
        brec = colp.tile([P, NCH], F32, tag="brec", bufs=1, name="brec")
        nc.vector.reciprocal(brec[:], beta_all[:])
        wg1m = colp.tile([P, NCH], F32, tag="wg1m", bufs=1, name="wg1m")
        nc.vector.tensor_scalar(wg1m[:], wg_c[:], -1.0, 1.0, op0=ALU.mult, op1=ALU.add)
        # folded gate columns: om = wg*o + (1-wg)/beta*vb + (1-wg)p1*ls + (1-wg)p2*ll
        b_c = colp.tile([P, NCH], F32, tag="b_c", bufs=1, name="b_c")
        nc.vector.tensor_tensor(b_c[:], wg1m[:], brec[:], op=ALU.mult)
        c1_c = colp.tile([P, NCH], F32, tag="c1_c", bufs=1, name="c1_c")
        nc.vector.tensor_tensor(c1_c[:], wg1m[:], p1[:], op=ALU.mult)
        c2_c = colp.tile([P, NCH], F32, tag="c2_c", bufs=1, name="c2_c")
        nc.vector.tensor_tensor(c2_c[:], wg1m[:], p2[:], op=ALU.mult)

        # ================= phase B: per-chunk scan =================
        def vtok_col(c):
            return vwin[:, :, FGUARD + c * CH: FGUARD + (c + 1) * CH]

        # ---- software-pipelined chunk stages ----
        # S1(c): token-major stats + G  (emitted 3 iterations ahead)
        # S2(c): scalings N1/Mt/Nt/attnT/khat/khatb  (2 ahead)
        # S3(c): Neumann ladder + P-chain + wTn/u0  (1 ahead)
        # S4(c): serial scan step  (current)
        # FIR-long matmuls are dripped between dependent S3 steps so the
        # in-order PE queue always has ready work.
        st1, st2, st3 = {}, {}, {}

        def S1(c):
            beta_c = beta_all[:, c:c + 1]
            res = {"beta": beta_c}
            tpv = ps_t.tile([P, 2, CH], F16, tag="pst", name="tp_v")
            for pt in range(2):
                nc.tensor.transpose(tpv[:, pt, :],
                                    vwin[:, pt, FGUARD + c * CH: FGUARD + (c + 1) * CH],
                                    ident[:])
            nc.vector.tensor_scalar_mul(vb_all[:, c, :], tpv[:], beta_c)
            res["kTsl"] = [rollkq[:, pt, c, 0, :] for pt in range(2)]
            res["qTsl"] = [rollkq[:, pt, c, 1, :] for pt in range(2)]
            res["kqTsl"] = [rollkq[:, pt, c, :, :] for pt in range(2)]
            toks = {}
            for t, koff in (("q", 1), ("k", 0)):
                tok = chk.tile([P, DV], F16, tag=f"{t}tok", bufs=3, name=f"{t}tok")
                tpt = ps_t.tile([P, 2, CH], F16, tag="pst", name=f"tp_{t}")
                for pt in range(2):
                    nc.tensor.transpose(tpt[:, pt, :], rollkq[:, pt, c, koff, :], ident[:])
                nc.scalar.copy(tok[:], tpt[:])
                toks[t] = tok
            res["ktok"] = toks["k"]
            # [Graw | Braw] = kraw @ [kraw | qraw]^T ; evict fp16 to free the bank
            gps = ps_med.tile([P, DV], F32, tag="psm", name="gps")
            for pt in range(2):
                nc.tensor.matmul(gps[:], res["kTsl"][pt], res["kqTsl"][pt],
                                 start=(pt == 0), stop=(pt == 1))
            g16 = chk.tile([P, DV], F16, tag="g16", bufs=3, name="g16")
            nc.scalar.copy(g16[:], gps[:])
            res["g16"] = g16
            for t in ("q", "k"):
                scr = chk.tile([P, DV], F16, tag="sq_scr", bufs=2, name="sq_scr")
                ssq = chk.tile([P, 1], F32, tag=f"ssq{t}", bufs=2, name=f"ssq{t}")
                nc.vector.scalar_tensor_tensor(scr[:], toks[t][:], 1.0, toks[t][:],
                                               op0=ALU.mult, op1=ALU.mult, accum_out=ssq[:])
                sroot = chk.tile([P, 1], F32, tag=f"sroot{t}", bufs=2, name=f"sroot{t}")
                nc.scalar.activation(sroot[:], ssq[:], AF.Sqrt, bias=eps_l2[:])
                rinv = chk.tile([P, 1], F32, tag=f"rinv{t}", bufs=5, name=f"rinv{t}")
                nc.vector.reciprocal(rinv[:], sroot[:])
                res["rinv" + t] = rinv
            st1[c] = res

        def S2(c, drip):
            res = st1.pop(c)
            beta_c, rinvk = res["beta"], res["rinvk"]
            khat = chk.tile([P, DV], F16, tag="khat", bufs=4, name="khat")
            nc.vector.tensor_scalar_mul(khat[:], res["ktok"][:], rinvk[:])
            res["khat"] = khat
            khatb = chk.tile([P, DV], F16, tag="khatb", bufs=3, name="khatb")
            nc.vector.tensor_scalar_mul(khatb[:], khat[:], beta_c)
            res["khatb"] = khatb
            br = chk.tile([P, 1], F32, tag="br", bufs=2, name="br")
            nc.vector.tensor_tensor(br[:], beta_c, rinvk[:], op=ALU.mult)
            N1 = chk.tile([P, P], F16, tag="N1", bufs=2, name="N1")
            nc.vector.scalar_tensor_tensor(N1[:], res["g16"][:, :P], br[:], masklt[:],
                                           op0=ALU.mult, op1=ALU.mult)
            drip(2)
            mps = ps_t.tile([P, 2, CH], F16, tag="pst", name="mps")
            nc.tensor.transpose(mps[:, 0, :], N1[:], ident[:])
            Mt = chk.tile([P, P], F16, tag="Mt", bufs=3, name="Mt")
            nc.vector.tensor_scalar_mul(Mt[:], mps[:, 0, :], rinvk[:])
            drip(2)
            nc.tensor.transpose(mps[:, 1, :], Mt[:], ident[:])
            Nt = chk.tile([P, P], F16, tag="Nt", bufs=3, name="Nt")
            nc.scalar.copy(Nt[:], mps[:, 1, :])
            res["Mt"], res["Nt"] = Mt, Nt
            attnT = chk.tile([P, P], F16, tag="attnT", bufs=4, name="attnT")
            nc.vector.scalar_tensor_tensor(attnT[:], res["g16"][:, P:], rinvk[:], maskut[:],
                                           op0=ALU.mult, op1=ALU.mult)
            res["attnT"] = attnT
            st2[c] = res

        def mm_small(lhsT, rhs, name, evict_eng, ps=None, half=0):
            if ps is None:
                ps = ps_med.tile([P, DV], F32, tag="psm", name=f"ps_{name}")
            psl = ps[:, half * P:(half + 1) * P]
            nc.tensor.matmul(psl, lhsT, rhs, start=True, stop=True)
            sb = chk.tile([P, P], F16, tag=name, bufs=2, name=name)
            if evict_eng == "v":
                nc.vector.tensor_copy(sb[:], psl)
            else:
                nc.scalar.copy(sb[:], psl)
            return sb, ps

        def S3(c, drip):
            res = st2.pop(c)
            Mt, Nt = res["Mt"], res["Nt"]
            # Neumann truncated at A^15: T^T = (I-Mt)(I+Mt^2)(I+Mt^4)(I+Mt^8)
            P1 = chk.tile([P, P], F16, tag="P1", bufs=2, name="P1")
            nc.vector.tensor_tensor(P1[:], ident[:], Mt[:], op=ALU.subtract)
            N2, ps2 = mm_small(Mt[:], Nt[:], "N2", "v")
            M2, _ = mm_small(Nt[:], Mt[:], "M2", "s", ps=ps2, half=1)
            drip(2)
            N4, ps4 = mm_small(M2[:], N2[:], "N4", "s")
            M4, _ = mm_small(N2[:], M2[:], "M4", "s", ps=ps4, half=1)
            drip(2)
            N8, _ = mm_small(M4[:], N4[:], "N8", "v")
            drip(2)
            Pc = P1
            for Npow, nm in ((N2, "P2"), (N4, "P3"), (N8, "TTt")):
                pps = ps_med.tile([P, DV], F32, tag="psm", name=f"pps_{nm}")
                nc.tensor.matmul(pps[:, :P], Npow[:], Pc[:], start=True, stop=True)
                nxt = chk.tile([P, P], F16, tag=nm, bufs=2, name=nm)
                nc.vector.tensor_tensor(nxt[:], Pc[:], pps[:, :P], op=ALU.add)
                Pc = nxt
                drip(2)
            TTt = Pc
            negTT = chk.tile([P, P], F16, tag="negTT", bufs=2, name="negTT")
            nc.vector.tensor_scalar_mul(negTT[:], TTt[:], -1.0)
            drip(2)
            wTn = chk.tile([P, 2, CH], F16, tag="wTn", bufs=3, name="wTn")
            wps = ps_med.tile([P, DV], F32, tag="psm", name="wps")
            for kt in range(2):
                nc.tensor.matmul(wps[:, kt * P:(kt + 1) * P],
                                 res["khatb"][:, kt * P:(kt + 1) * P], negTT[:],
                                 start=True, stop=True)
            nc.scalar.copy(wTn[:], wps[:])
            res["wTn"] = wTn
            ups = ps_med.tile([P, DV], F32, tag="psm", name="ups")
            nc.tensor.matmul(ups[:], TTt[:], vb_all[:, c, :], start=True, stop=True)
            u0 = chk.tile([P, DV], F16, tag="u0", bufs=3, name="u0")
            nc.scalar.copy(u0[:], ups[:])
            res["u0"] = u0
            st3[c] = res

        def S4(c):
            pr = st3.pop(c)
            Sold, Snew = S16[(c + 1) % 2], S16[c % 2]
            if c == 0:
                u16 = pr["u0"]
            else:
                ups = ps_med.tile([P, DV], F32, tag="psm", name="ups_s")
                for kt in range(2):
                    nc.tensor.matmul(ups[:], pr["wTn"][:, kt, :], Sold[:, kt, :],
                                     start=(kt == 0), stop=(kt == 1))
                u16 = chk.tile([P, DV], F16, tag="u16", bufs=2, name="u16")
                nc.vector.tensor_tensor(u16[:], ups[:], pr["u0"][:], op=ALU.add)
            # S update first: it is the cross-chunk critical path
            # (S(c-1) -> ups -> u16 -> dps -> S(c)); o can fill in after
            dps = ps_big.tile([P, 2, DV], F32, tag="psb", name="dps")
            for kt in range(2):
                nc.tensor.matmul(dps[:, kt, :], pr["khat"][:, kt * P:(kt + 1) * P], u16[:],
                                 start=True, stop=True)
            if c == 0:
                nc.vector.tensor_copy(Snew[:], dps[:])
            else:
                nc.vector.tensor_tensor(Snew[:], Sold[:], dps[:], op=ALU.add)
            ops = ps_med.tile([P, DV], F32, tag="psm", name="ops")
            if c == 0:
                nc.tensor.matmul(ops[:], pr["attnT"][:], u16[:], start=True, stop=True)
            else:
                for kt in range(2):
                    nc.tensor.matmul(ops[:], pr["qTsl"][kt], Sold[:, kt, :],
                                     start=(kt == 0), stop=False)
                nc.tensor.matmul(ops[:], pr["attnT"][:], u16[:], start=False, stop=True)
            nc.scalar.activation(o_all[:, c, :], ops[:], AF.Identity, scale=pr["rinvq"][:])

        # ---- FIR long: per-iteration batches of PE matmuls, dripped via S3 ----
        fir_state = {}
        fch_of = {}   # n -> fch dict

        def fir_mm_thunks(k):
            """Iteration k (0..NCH-1): list of closures, one FIR matmul each."""
            n, b = k // CPN, k % CPN
            pt, half = b // 2, b % 2
            st = fir_state.setdefault(n, {})
            thunks = []
            if half == 0:
                ps = ps_fir.tile([P, NTILE], F32, tag="fir", name="ps_ll")
                st[pt] = ps
                taps = range(0, 16)
            else:
                ps = st[pt]
                taps = range(16, FIR_L)
            for j in taps:
                def mk(j=j, ps=ps, pt=pt, n=n):
                    nc.tensor.matmul(ps[:], fdiag[:, pt, j, :],
                                     vwin[:, pt, n * NTILE + 1 + j: n * NTILE + 1 + j + NTILE],
                                     start=(j == 0), stop=(j == FIR_L - 1))
                thunks.append(mk)
            return thunks

        def fir_finish(k):
            """Evicts + FIR-short chain for iteration k's batch."""
            n, b = k // CPN, k % CPN
            pt, half = b // 2, b % 2
            fch = fch_of.setdefault(n, {})
            if half == 1:
                ll = gat.tile([P, NTILE], F16, tag="llch", bufs=6, name="llch")
                nc.scalar.copy(ll[:], fir_state[n][pt][:])
                fch[("ll", pt)] = ll
            else:
                ls = gat.tile([P, NTILE], F16, tag="lsch", bufs=6, name="lsch")
                bs = FGUARD - FIR_S + 1 + n * NTILE
                nc.vector.tensor_scalar_mul(ls[:], vwin[:, pt, bs:bs + NTILE],
                                            staps[:, pt, 0:1])
                for j in range(1, FIR_S):
                    sprod = gat.tile([P, NTILE], F16, tag="sprod", bufs=2, name="sprod")
                    nc.vector.tensor_scalar_mul(sprod[:], vwin[:, pt, bs + j:bs + j + NTILE],
                                                staps[:, pt, j:j + 1])
                    nc.vector.tensor_tensor(ls[:], ls[:], sprod[:], op=ALU.add)
                fch[("ls", pt)] = ls

        # ================= phase C: gating + output projection =================
        def gate_out(lt, fch):
            off = (lt % CPN) * CH
            cs = lambda t: t[:, lt:lt + 1]
            toks = {}
            for f in ("ls", "ll"):
                tokt = gat.tile([P, DV], F16, tag=f"{f}tok", bufs=2, name=f"{f}tok")
                tp = ps_t.tile([P, 2, CH], F16, tag="pst", name=f"tp_{f}")
                for pt in range(2):
                    nc.tensor.transpose(tp[:, pt, :], fch[(f, pt)][:, off:off + CH], ident[:])
                nc.scalar.copy(tokt[:], tp[:])
                toks[f] = tokt
            # folded gating: 4 DVE tensor_scalar products (2x mode) + 3 Pool adds
            po = gat.tile([P, DV], F16, tag="gtmp", bufs=8, name="po")
            nc.vector.tensor_scalar_mul(po[:], o_all[:, lt, :], cs(wg_c))
            pv = gat.tile([P, DV], F16, tag="gtmp", bufs=8, name="pv")
            nc.vector.tensor_scalar_mul(pv[:], vb_all[:, lt, :], cs(b_c))
            pls = gat.tile([P, DV], F16, tag="gtmp", bufs=8, name="pls")
            nc.vector.tensor_scalar_mul(pls[:], toks["ls"][:], cs(c1_c))
            pll = gat.tile([P, DV], F16, tag="gtmp", bufs=8, name="pll")
            nc.vector.tensor_scalar_mul(pll[:], toks["ll"][:], cs(c2_c))
            s1g = gat.tile([P, DV], F16, tag="gtmp", bufs=8, name="s1g")
            nc.gpsimd.tensor_tensor(s1g[:], po[:], pv[:], op=ALU.add)
            s2g = gat.tile([P, DV], F16, tag="gtmp", bufs=8, name="s2g")
            nc.gpsimd.tensor_tensor(s2g[:], pls[:], pll[:], op=ALU.add)
            om = gat.tile([P, DV], F16, tag="gtmp", bufs=8, name="om")
            nc.gpsimd.tensor_tensor(om[:], s1g[:], s2g[:], op=ALU.add)
            scr = gat.tile([P, DV], F16, tag="scr_g", bufs=2, name="scr_g")
            ssq = gat.tile([P, 1], F32, tag="ssq_g", bufs=2, name="ssq_g")
            nc.vector.scalar_tensor_tensor(scr[:], om[:], 1.0, om[:],
                                           op0=ALU.mult, op1=ALU.mult, accum_out=ssq[:])
            srt = gat.tile([P, 1], F32, tag="srt_g", bufs=2, name="srt_g")
            nc.scalar.activation(srt[:], ssq[:], AF.Sqrt, bias=eps_rms[:], scale=1.0 / DV)
            rin = gat.tile([P, 1], F32, tag="rin_g", bufs=2, name="rin_g")
            nc.vector.reciprocal(rin[:], srt[:])
            # output projection: transpose unnormalized om; fold the RMS scale
            # into the PSUM eviction (per-partition scale on out rows = tokens)
            onT = gat.tile([P, 2, CH], F16, tag="onT", bufs=2, name="onT")
            tp = ps_t.tile([P, 2, CH], F16, tag="pst", name="tp_on")
            for pt in range(2):
                nc.tensor.transpose(tp[:, pt, :], om[:, pt * P:(pt + 1) * P], ident[:])
            nc.scalar.copy(onT[:], tp[:])
            out_sb = gat.tile([P, D], F16, tag="out_sb", bufs=2, name="out_sb")
            for nt2 in range(2):
                ops = ps_big.tile([P, NTILE], F32, tag="psb", name="ops_o")
                for kt in range(2):
                    nc.tensor.matmul(ops[:], onT[:, kt, :],
                                     wo[:, kt, nt2 * NTILE:(nt2 + 1) * NTILE],
                                     start=(kt == 0), stop=(kt == 1))
                nc.scalar.activation(out_sb[:, nt2 * NTILE:(nt2 + 1) * NTILE], ops[:],
                                     AF.Identity, scale=rin[:])
            nc.sync.dma_start(out_d[lt * CH:(lt + 1) * CH, :], out_sb[:])

        # ---- steady-state emission: stages offset by iteration ----
        for it in range(NCH + 4):
            c4, c3, c2, c1 = it - 3, it - 2, it - 1, it
            gl = it - 4
            pend = list(fir_mm_thunks(it)) if it < NCH else []

            def drip(k):
                for _ in range(min(k, len(pend))):
                    pend.pop(0)()

            if 0 <= c4 < NCH:
                S4(c4)
            drip(2)
            if 0 <= c2 < NCH:
                S2(c2, drip)
            if 0 <= c3 < NCH:
                S3(c3, drip)
            drip(len(pend))
            if it < NCH:
                fir_finish(it)
            if 0 <= c1 < NCH:
                S1(c1)
            if 0 <= gl < NCH:
                gate_out(gl, fch_of[gl // CPN])

    nc.compile()
    return nc


# ---------------- host side ----------------

def _diag_stack(taps):
    """taps [C, K] -> [2, K, 128, 128] diag matrices."""
    C, K = taps.shape
    out = np.zeros((2, K, P, P), np.float32)
    for pt in range(2):
        for j in range(K):
            np.fill_diagonal(out[pt, j], taps[pt * P:(pt + 1) * P, j])
    return out


def make_core_inputs(inputs, b, h, L):
    f = lambda a: np.ascontiguousarray(np.asarray(a, np.float32))
    x = f(inputs['hidden_states'])[b]          # [L, D]
    temp = float(np.exp(np.asarray(inputs['log_temp'], np.float64)[h]))
    wsm = np.concatenate([
        f(inputs['Wb'])[:, h:h + 1],
        f(inputs['Wg'])[:, h:h + 1],
        f(inputs['Wl'])[:, 3 * h:3 * h + 3] / temp], axis=1)
    bias5 = np.array([0.0, float(np.asarray(inputs['bg'], np.float64)[h]),
                      *(np.asarray(inputs['bl'], np.float64)[3 * h:3 * h + 3] / temp)],
                     np.float32)
    ct = np.stack([
        f(inputs['conv_q'])[h * DK:(h + 1) * DK].reshape(2, P, CONV_K),
        f(inputs['conv_k'])[h * DK:(h + 1) * DK].reshape(2, P, CONV_K),
        f(inputs['conv_v'])[h * DV:(h + 1) * DV].reshape(2, P, CONV_K)])  # [3, 2, 128, 4]
    # residual FIR taps: fir = delta + r  ->  local = v + FIR_r(v); softmax sums to 1
    fs = f(inputs['fir_short'])[h].copy()   # [DV, 3]
    fs[:, -1] -= 1.0
    fl = f(inputs['fir_long'])[h].copy()    # [DV, 31]
    fl[:, -1] -= 1.0
    fd = _diag_stack(fl).astype(np.float16)
    st = fs.reshape(2, P, FIR_S)
    wo = f(inputs['rms_w'])[:, None] * f(inputs['Wo'])[h * DV:(h + 1) * DV]
    h16 = np.float16
    return dict(
        xT=np.ascontiguousarray(x.T).astype(h16),
        wq=np.ascontiguousarray(f(inputs['Wq'])[:, h * DK:(h + 1) * DK]).astype(h16),
        wk=np.ascontiguousarray(f(inputs['Wk'])[:, h * DK:(h + 1) * DK]).astype(h16),
        wv=np.ascontiguousarray(f(inputs['Wv'])[:, h * DV:(h + 1) * DV]).astype(h16),
        wsm=wsm.astype(h16), bias5=bias5,
        ctaps=ct, fdiag=fd, staps=st.astype(np.float32), wo=wo.astype(h16),
        ident=np.eye(P, dtype=h16),
        masklt=np.tril(np.ones((P, P), h16), -1),
        maskut=np.triu(np.ones((P, P), h16), 0),
    )


_NC_CACHE = {}


def _get_nc(L):
    if L not in _NC_CACHE:
        _NC_CACHE[L] = build(L)
    return _NC_CACHE[L]


def kernel(**inputs):
    x = np.asarray(inputs['hidden_states'])
    Bx, L, _ = x.shape
    nc = _get_nc(L)
    in_maps = [make_core_inputs(inputs, c // H, c % H, L) for c in range(8)]
    res = run_bass_kernel_spmd(nc, in_maps, core_ids=list(range(8)))
    out = np.zeros((Bx, L, D), np.float32)
    for c in range(8):
        out[c // H] += res.results[c]['out'].astype(np.float32)
    return out


# revision 62
# speedup vs baseline: 1.0408x; 1.0408x over previous
"""DeltaNet Bass kernel for Trainium2, 8-core SPMD — fp16 matmul pipeline.

Sharding: core = (b, h) for b in 0..1, h in 0..3  (b*4 + h).
Each core computes the full per-(batch,head) pipeline and its partial
output projection out_partial[L, D] in fp16; the host sums the 4
head-partials per batch.

Device pipeline per core (PSUM accumulate fp32, operands fp16):
  phase A (per 512-col tile, all 8 tiles first): q/k/v/small projections
          (PE, fp16), causal 4-tap conv (DVE STT chain, fp16 4x mode),
          SiLU evict (ACT) into persistent SBUF: rollkq (chan-major k|q
          interleaved per 128-token chunk) and vwin (guarded chan-major v).
  phase B prologue: beta + gate columns from rows scratch (one Sigmoid /
          Exp table load each).
  phase B (per 128-token chunk): PE transposes (fp16 PSUM), l2norm stats,
          UT-transform T^T via Neumann product, chunk-local prepass,
          serial scan (u = u0 - w S, o = q S + attn u, S += k^T u) with
          fp32 S master + fp16 S copy; FIR-long diag matmuls + FIR-short
          STT chains interleaved per tile to fill PE gaps.
  phase C (per chunk, pipelined one tile behind): FIR transposes,
          hierarchical gating (DVE fp16), RMSNorm, output projection.
"""
import numpy as np
from contextlib import ExitStack

import concourse.bass as bass
import concourse.tile as tile
from concourse import bacc, mybir
from concourse.bass_utils import run_bass_kernel_spmd

F32 = mybir.dt.float32
F16 = mybir.dt.float16
AF = mybir.ActivationFunctionType
ALU = mybir.AluOpType

B, D, H, DK, DV = 2, 1024, 4, 256, 256
CONV_K, FIR_S, FIR_L = 4, 3, 31
CH = 128          # scan chunk (token tile)
NTILE = 512       # column tile for projections / FIR
P = 128
KT = D // P       # 8 contraction tiles over D
GUARD = CONV_K - 1
EPS_RMS = 1e-5
FGUARD = FIR_L    # guard cols ahead of token 0 in vwin


def build(L=4096):
    NT = L // NTILE
    NCH = L // CH
    CPN = NTILE // CH  # chunks per n-tile (4)

    nc = bacc.Bacc("TRN2", target_bir_lowering=False, debug=False, num_devices=8)

    xT_d = nc.dram_tensor("xT", [D, L], F16, kind="ExternalInput").ap()
    wq_d = nc.dram_tensor("wq", [D, DK], F16, kind="ExternalInput").ap()
    wk_d = nc.dram_tensor("wk", [D, DK], F16, kind="ExternalInput").ap()
    wv_d = nc.dram_tensor("wv", [D, DV], F16, kind="ExternalInput").ap()
    wsm_d = nc.dram_tensor("wsm", [D, 5], F16, kind="ExternalInput").ap()
    bias5_d = nc.dram_tensor("bias5", [5], F32, kind="ExternalInput").ap()
    # conv taps per (tensor, pt): [3, 2, 128, 4]
    ctaps_d = nc.dram_tensor("ctaps", [3, 2, P, CONV_K], F32, kind="ExternalInput").ap()
    # fir long-residual diagonal matrices: [pt=2, 31, 128, 128]
    fdiag_d = nc.dram_tensor("fdiag", [2, FIR_L, P, P], F16, kind="ExternalInput").ap()
    # fir short-residual taps: [2, 128, 3]
    staps_d = nc.dram_tensor("staps", [2, P, FIR_S], F32, kind="ExternalInput").ap()
    wo_d = nc.dram_tensor("wo", [DV, D], F16, kind="ExternalInput").ap()
    ident_d = nc.dram_tensor("ident", [P, P], F16, kind="ExternalInput").ap()
    masklt_d = nc.dram_tensor("masklt", [P, P], F16, kind="ExternalInput").ap()  # strict lower
    maskut_d = nc.dram_tensor("maskut", [P, P], F16, kind="ExternalInput").ap()  # upper incl diag
    out_d = nc.dram_tensor("out", [L, D], F16, kind="ExternalOutput").ap()

    with tile.TileContext(nc) as tc, ExitStack() as ctx:
        # ---------------- pools ----------------
        const = ctx.enter_context(tc.tile_pool(name="const", bufs=1))
        bigw = ctx.enter_context(tc.tile_pool(name="bigw", bufs=1))
        persist = ctx.enter_context(tc.tile_pool(name="persist", bufs=1))
        xtp = ctx.enter_context(tc.tile_pool(name="xtp", bufs=2))
        prep = ctx.enter_context(tc.tile_pool(name="prep", bufs=1))
        colp = ctx.enter_context(tc.tile_pool(name="colp", bufs=1))
        chk = ctx.enter_context(tc.tile_pool(name="chk", bufs=1))
        sp = ctx.enter_context(tc.tile_pool(name="sp", bufs=1))
        gat = ctx.enter_context(tc.tile_pool(name="gat", bufs=1))
        dram = ctx.enter_context(tc.tile_pool(name="dram", bufs=1, space="DRAM"))
        ps_big = ctx.enter_context(tc.tile_pool(name="ps_big", bufs=2, space="PSUM"))
        ps_med = ctx.enter_context(tc.tile_pool(name="ps_med", bufs=3, space="PSUM"))
        ps_t = ctx.enter_context(tc.tile_pool(name="ps_t", bufs=2, space="PSUM"))
        ps_fir = ctx.enter_context(tc.tile_pool(name="ps_fir", bufs=1, space="PSUM"))



        # ---------------- constants / weights ----------------
        wq = bigw.tile([P, KT, DK], F16, tag="wq")
        nc.sync.dma_start(wq[:], wq_d.rearrange("(kt p) m -> p kt m", p=P))
        wk = bigw.tile([P, KT, DK], F16, tag="wk")
        nc.sync.dma_start(wk[:], wk_d.rearrange("(kt p) m -> p kt m", p=P))
        wv = bigw.tile([P, KT, DV], F16, tag="wv")
        nc.sync.dma_start(wv[:], wv_d.rearrange("(kt p) m -> p kt m", p=P))
        wsm = const.tile([P, KT, 5], F16)
        nc.sync.dma_start(wsm[:], wsm_d.rearrange("(kt p) m -> p kt m", p=P))
        ident = const.tile([P, P], F16)
        nc.sync.dma_start(ident[:], ident_d)
        masklt = const.tile([P, P], F16)
        nc.sync.dma_start(masklt[:], masklt_d)
        maskut = const.tile([P, P], F16)
        nc.sync.dma_start(maskut[:], maskut_d)
        bias5 = const.tile([5, 1], F32)
        nc.sync.dma_start(bias5[:], bias5_d.rearrange("(m o) -> m o", o=1))
        ctaps = const.tile([P, 3, 2, CONV_K], F32, name="ctaps")
        nc.sync.dma_start(ctaps[:], ctaps_d.rearrange("t pt p j -> p t pt j"))
        staps = const.tile([P, 2, FIR_S], F32, name="staps")
        nc.sync.dma_start(staps[:], staps_d.rearrange("pt p j -> p pt j"))
        fdiag = bigw.tile([P, 2, FIR_L, P], F16, tag="fdiag", name="fdiag")
        nc.sync.dma_start(fdiag[:], fdiag_d.rearrange("pt j p q -> p pt j q"))
        wo = bigw.tile([P, 2, D], F16, tag="wo", name="wo")
        nc.sync.dma_start(wo[:], wo_d.rearrange("(kt p) m -> p kt m", p=P))
        zeros3 = const.tile([P, GUARD], F16)
        nc.vector.memset(zeros3[:], 0.0)
        eps_l2 = const.tile([P, 1], F32)
        nc.vector.memset(eps_l2[:], 1e-6)
        eps_rms = const.tile([P, 1], F32)
        nc.vector.memset(eps_rms[:], EPS_RMS)

        # ---------------- persistent SBUF state ----------------
        # gate columns: [P, chunk, (beta_raw, wg_raw, l0, l1, l2, pad*3)] token-major
        gcols = persist.tile([P, NCH, 8], F16, name="gcols")
        # chan-major post-silu k|q interleaved per chunk: [P, pt, chunk, (k|q), CH]
        rollkq = persist.tile([P, 2, NCH, 2, CH], F16, name="rollkq")
        # chan-major post-silu v with FGUARD leading guard cols
        vwin = persist.tile([P, 2, FGUARD + L], F16, name="vwin")
        nc.vector.memset(vwin[:, :, 0:FGUARD], 0.0)
        # per-chunk outputs of the scan + beta-scaled v (token-major)
        o_all = persist.tile([P, NCH, DV], F16, name="o_all")
        vb_all = persist.tile([P, NCH, DV], F16, name="vb_all")

        S16 = [sp.tile([P, 2, DV], F16, name=f"S16_{i}") for i in range(2)]

        TENS = ("q", "k", "v")
        W_OF = {"q": wq, "k": wk, "v": wv}
        prev_pre = {}

        # ================= phase A: projections + conv + silu =================
        def phaseA(n):
            xt = xtp.tile([P, KT, NTILE], F16, tag="xt", name="xt")
            nc.sync.dma_start(xt[:], xT_d[:, n * NTILE:(n + 1) * NTILE]
                              .rearrange("(kt p) t -> p kt t", p=P))
            for t in TENS:
                for pt in range(2):
                    ps = ps_big.tile([P, NTILE], F32, tag="psb", name=f"ps_{t}{pt}")
                    for kt in range(KT):
                        nc.tensor.matmul(ps[:], W_OF[t][:, kt, pt * P:(pt + 1) * P],
                                         xt[:, kt, :], start=(kt == 0), stop=(kt == KT - 1))
                    key = f"pre{t}{pt}"
                    pre = prep.tile([P, GUARD + NTILE], F16, tag=key, bufs=2, name=key)
                    if n == 0:
                        nc.scalar.copy(pre[:, 0:GUARD], zeros3[:])
                    else:
                        nc.scalar.copy(pre[:, 0:GUARD], prev_pre[key][:, NTILE:NTILE + GUARD])
                    nc.scalar.copy(pre[:, GUARD:], ps[:])
                    prev_pre[key] = pre
                    # conv: 4-tap chain. q/k on DVE in TS+TT form (tensor_scalar
                    # and tensor_tensor hit the 2x 16-bit DVE mode; STT does
                    # not). v on the idle Pool engine (TT-only, broadcast taps).
                    ti = TENS.index(t)
                    acc = prep.tile([P, NTILE], F16, tag="cacc", bufs=2, name="cacc")
                    if t == "v" and pt == 1:
                        bct = lambda j: ctaps[:, ti, pt, j:j + 1].to_broadcast((P, NTILE))
                        nc.gpsimd.tensor_tensor(acc[:], pre[:, 0:NTILE], bct(0), op=ALU.mult)
                        for j in range(1, CONV_K):
                            prod = prep.tile([P, NTILE], F16, tag="cprod", bufs=2, name="cprod")
                            nc.gpsimd.tensor_tensor(prod[:], pre[:, j:j + NTILE], bct(j),
                                                    op=ALU.mult)
                            nc.gpsimd.tensor_tensor(acc[:], acc[:], prod[:], op=ALU.add)
                    else:
                        nc.vector.tensor_scalar_mul(acc[:], pre[:, 0:NTILE],
                                                    ctaps[:, ti, pt, 0:1])
                        for j in range(1, CONV_K):
                            prod = prep.tile([P, NTILE], F16, tag="cprod", bufs=2, name="cprod")
                            nc.vector.tensor_scalar_mul(prod[:], pre[:, j:j + NTILE],
                                                        ctaps[:, ti, pt, j:j + 1])
                            nc.vector.tensor_tensor(acc[:], acc[:], prod[:], op=ALU.add)
                    # silu -> persistent layout
                    if t == "v":
                        nc.scalar.activation(
                            vwin[:, pt, FGUARD + n * NTILE: FGUARD + (n + 1) * NTILE],
                            acc[:], AF.Silu)
                    else:
                        koff = 0 if t == "k" else 1
                        nc.scalar.activation(
                            rollkq[:, pt, n * CPN:(n + 1) * CPN, koff, :],
                            acc[:], AF.Silu)
            # small projections [5, NTILE]
            ps5 = ps_big.tile([P, NTILE], F32, tag="psb", name="ps5")
            for kt in range(KT):
                nc.tensor.matmul(ps5[:5, :], wsm[:, kt, :], xt[:, kt, :],
                                 start=(kt == 0), stop=(kt == KT - 1))
            rows_sb = colp.tile([5, NTILE], F16, tag="rows_sb", bufs=2, name="rows_sb")
            nc.scalar.activation(rows_sb[:], ps5[:5, :], AF.Identity, bias=bias5[:])
            # token-major gate columns via tiny PE transposes (no DRAM round trip)
            rtp = ps_t.tile([P, 2, CH], F16, tag="pst", name="rtp")
            for ci in range(CPN):
                nc.tensor.transpose(rtp[:, 0, ci * 8:ci * 8 + 5],
                                    rows_sb[:, ci * CH:(ci + 1) * CH], ident[:5, :5])
            nc.scalar.copy(gcols[:, n * CPN:(n + 1) * CPN, :], rtp[:, 0, 0:8 * CPN])

        for n in range(NT):
            phaseA(n)

        # ================= phase B prologue: beta + gate columns =================
        beta_all = colp.tile([P, NCH], F32, tag="beta_all", bufs=1, name="beta_all")
        nc.scalar.activation(beta_all[:], gcols[:, :, 0], AF.Sigmoid)
        wg_c = colp.tile([P, NCH], F32, tag="wg_c", bufs=1, name="wg_c")
        nc.scalar.activation(wg_c[:], gcols[:, :, 1], AF.Sigmoid)
        mx = colp.tile([P, NCH], F32, tag="mx", bufs=1, name="mx")
        nc.vector.tensor_tensor(mx[:], gcols[:, :, 2], gcols[:, :, 3], op=ALU.max)
        nc.vector.tensor_tensor(mx[:], mx[:], gcols[:, :, 4], op=ALU.max)
        e0 = colp.tile([P, NCH], F32, tag="e0", bufs=1, name="e0")
        e1 = colp.tile([P, NCH], F32, tag="e1", bufs=1, name="e1")
        e2 = colp.tile([P, NCH], F32, tag="e2", bufs=1, name="e2")
        for r, dst in ((2, e0), (3, e1), (4, e2)):
            nc.vector.tensor_tensor(dst[:], gcols[:, :, r], mx[:], op=ALU.subtract)
            nc.scalar.activation(dst[:], dst[:], AF.Exp)
        esum = colp.tile([P, NCH], F32, tag="esum", bufs=1, name="esum")
        nc.vector.tensor_tensor(esum[:], e0[:], e1[:], op=ALU.add)
        nc.vector.tensor_tensor(esum[:], esum[:], e2[:], op=ALU.add)
        erec = colp.tile([P, NCH], F32, tag="erec", bufs=1, name="erec")
        nc.vector.reciprocal(erec[:], esum[:])
        p1 = colp.tile([P, NCH], F32, tag="p1", bufs=1, name="p1")
        p2 = colp.tile([P, NCH], F32, tag="p2", bufs=1, name="p2")
        for src, dst in ((e1, p1), (e2, p2)):
            nc.vector.tensor_tensor(dst[:], src[:], erec[:], op=ALU.mult)
        brec = colp.tile([P, NCH], F32, tag="brec", bufs=1, name="brec")
        nc.vector.reciprocal(brec[:], beta_all[:])
        wg1m = colp.tile([P, NCH], F32, tag="wg1m", bufs=1, name="wg1m")
        nc.vector.tensor_scalar(wg1m[:], wg_c[:], -1.0, 1.0, op0=ALU.mult, op1=ALU.add)
        # folded gate columns: om = wg*o + (1-wg)/beta*vb + (1-wg)p1*ls + (1-wg)p2*ll
        b_c = colp.tile([P, NCH], F32, tag="b_c", bufs=1, name="b_c")
        nc.vector.tensor_tensor(b_c[:], wg1m[:], brec[:], op=ALU.mult)
        c1_c = colp.tile([P, NCH], F32, tag="c1_c", bufs=1, name="c1_c")
        nc.vector.tensor_tensor(c1_c[:], wg1m[:], p1[:], op=ALU.mult)
        c2_c = colp.tile([P, NCH], F32, tag="c2_c", bufs=1, name="c2_c")
        nc.vector.tensor_tensor(c2_c[:], wg1m[:], p2[:], op=ALU.mult)

        # ================= phase B: per-chunk scan =================
        def vtok_col(c):
            return vwin[:, :, FGUARD + c * CH: FGUARD + (c + 1) * CH]

        # ---- software-pipelined chunk stages ----
        # S1(c): token-major stats + G  (emitted 3 iterations ahead)
        # S2(c): scalings N1/Mt/Nt/attnT/khat/khatb  (2 ahead)
        # S3(c): Neumann ladder + P-chain + wTn/u0  (1 ahead)
        # S4(c): serial scan step  (current)
        # FIR-long matmuls are dripped between dependent S3 steps so the
        # in-order PE queue always has ready work.
        st1, st2, st3 = {}, {}, {}

        def S1(c):
            beta_c = beta_all[:, c:c + 1]
            res = {"beta": beta_c}
            tpv = ps_t.tile([P, 2, CH], F16, tag="pst", name="tp_v")
            for pt in range(2):
                nc.tensor.transpose(tpv[:, pt, :],
                                    vwin[:, pt, FGUARD + c * CH: FGUARD + (c + 1) * CH],
                                    ident[:])
            nc.vector.tensor_scalar_mul(vb_all[:, c, :], tpv[:], beta_c)
            res["kTsl"] = [rollkq[:, pt, c, 0, :] for pt in range(2)]
            res["qTsl"] = [rollkq[:, pt, c, 1, :] for pt in range(2)]
            res["kqTsl"] = [rollkq[:, pt, c, :, :] for pt in range(2)]
            toks = {}
            for t, koff in (("q", 1), ("k", 0)):
                tok = chk.tile([P, DV], F16, tag=f"{t}tok", bufs=3, name=f"{t}tok")
                tpt = ps_t.tile([P, 2, CH], F16, tag="pst", name=f"tp_{t}")
                for pt in range(2):
                    nc.tensor.transpose(tpt[:, pt, :], rollkq[:, pt, c, koff, :], ident[:])
                nc.scalar.copy(tok[:], tpt[:])
                toks[t] = tok
            res["ktok"] = toks["k"]
            # [Graw | Braw] = kraw @ [kraw | qraw]^T ; evict fp16 to free the bank
            gps = ps_med.tile([P, DV], F32, tag="psm", name="gps")
            for pt in range(2):
                nc.tensor.matmul(gps[:], res["kTsl"][pt], res["kqTsl"][pt],
                                 start=(pt == 0), stop=(pt == 1))
            g16 = chk.tile([P, DV], F16, tag="g16", bufs=3, name="g16")
            nc.scalar.copy(g16[:], gps[:])
            res["g16"] = g16
            for t in ("q", "k"):
                scr = chk.tile([P, DV], F16, tag="sq_scr", bufs=2, name="sq_scr")
                ssq = chk.tile([P, 1], F32, tag=f"ssq{t}", bufs=2, name=f"ssq{t}")
                nc.vector.scalar_tensor_tensor(scr[:], toks[t][:], 1.0, toks[t][:],
                                               op0=ALU.mult, op1=ALU.mult, accum_out=ssq[:])
                sroot = chk.tile([P, 1], F32, tag=f"sroot{t}", bufs=2, name=f"sroot{t}")
                nc.scalar.activation(sroot[:], ssq[:], AF.Sqrt, bias=eps_l2[:])
                rinv = chk.tile([P, 1], F32, tag=f"rinv{t}", bufs=5, name=f"rinv{t}")
                nc.vector.reciprocal(rinv[:], sroot[:])
                res["rinv" + t] = rinv
            st1[c] = res

        def S2(c, drip):
            res = st1.pop(c)
            beta_c, rinvk = res["beta"], res["rinvk"]
            khat = chk.tile([P, DV], F16, tag="khat", bufs=4, name="khat")
            nc.vector.tensor_scalar_mul(khat[:], res["ktok"][:], rinvk[:])
            res["khat"] = khat
            khatb = chk.tile([P, DV], F16, tag="khatb", bufs=3, name="khatb")
            nc.vector.tensor_scalar_mul(khatb[:], khat[:], beta_c)
            res["khatb"] = khatb
            br = chk.tile([P, 1], F32, tag="br", bufs=2, name="br")
            nc.vector.tensor_tensor(br[:], beta_c, rinvk[:], op=ALU.mult)
            N1 = chk.tile([P, P], F16, tag="N1", bufs=2, name="N1")
            nc.vector.scalar_tensor_tensor(N1[:], res["g16"][:, :P], br[:], masklt[:],
                                           op0=ALU.mult, op1=ALU.mult)
            drip(2)
            mps = ps_t.tile([P, 2, CH], F16, tag="pst", name="mps")
            nc.tensor.transpose(mps[:, 0, :], N1[:], ident[:])
            Mt = chk.tile([P, P], F16, tag="Mt", bufs=3, name="Mt")
            nc.vector.tensor_scalar_mul(Mt[:], mps[:, 0, :], rinvk[:])
            drip(2)
            nc.tensor.transpose(mps[:, 1, :], Mt[:], ident[:])
            Nt = chk.tile([P, P], F16, tag="Nt", bufs=3, name="Nt")
            nc.scalar.copy(Nt[:], mps[:, 1, :])
            res["Mt"], res["Nt"] = Mt, Nt
            attnT = chk.tile([P, P], F16, tag="attnT", bufs=4, name="attnT")
            nc.vector.scalar_tensor_tensor(attnT[:], res["g16"][:, P:], rinvk[:], maskut[:],
                                           op0=ALU.mult, op1=ALU.mult)
            res["attnT"] = attnT
            st2[c] = res

        def mm_small(lhsT, rhs, name, evict_eng):
            ps = ps_med.tile([P, DV], F32, tag="psm", name=f"ps_{name}")
            nc.tensor.matmul(ps[:, :P], lhsT, rhs, start=True, stop=True)
            sb = chk.tile([P, P], F16, tag=name, bufs=2, name=name)
            if evict_eng == "v":
                nc.vector.tensor_copy(sb[:], ps[:, :P])
            else:
                nc.scalar.copy(sb[:], ps[:, :P])
            return sb

        def S3(c, drip):
            res = st2.pop(c)
            Mt, Nt = res["Mt"], res["Nt"]
            # Neumann truncated at A^15: T^T = (I-Mt)(I+Mt^2)(I+Mt^4)(I+Mt^8)
            P1 = chk.tile([P, P], F16, tag="P1", bufs=2, name="P1")
            nc.vector.tensor_tensor(P1[:], ident[:], Mt[:], op=ALU.subtract)
            N2 = mm_small(Mt[:], Nt[:], "N2", "v")
            M2 = mm_small(Nt[:], Mt[:], "M2", "s")
            drip(2)
            N4 = mm_small(M2[:], N2[:], "N4", "s")
            M4 = mm_small(N2[:], M2[:], "M4", "s")
            drip(2)
            N8 = mm_small(M4[:], N4[:], "N8", "v")
            drip(2)
            Pc = P1
            for Npow, nm in ((N2, "P2"), (N4, "P3"), (N8, "TTt")):
                pps = ps_med.tile([P, DV], F32, tag="psm", name=f"pps_{nm}")
                nc.tensor.matmul(pps[:, :P], Npow[:], Pc[:], start=True, stop=True)
                nxt = chk.tile([P, P], F16, tag=nm, bufs=2, name=nm)
                nc.vector.tensor_tensor(nxt[:], Pc[:], pps[:, :P], op=ALU.add)
                Pc = nxt
                drip(2)
            TTt = Pc
            negTT = chk.tile([P, P], F16, tag="negTT", bufs=2, name="negTT")
            nc.vector.tensor_scalar_mul(negTT[:], TTt[:], -1.0)
            drip(2)
            wTn = chk.tile([P, 2, CH], F16, tag="wTn", bufs=3, name="wTn")
            for kt in range(2):
                wps = ps_med.tile([P, DV], F32, tag="psm", name="wps")
                nc.tensor.matmul(wps[:, :P], res["khatb"][:, kt * P:(kt + 1) * P], negTT[:],
                                 start=True, stop=True)
                nc.scalar.copy(wTn[:, kt, :], wps[:, :P])
            res["wTn"] = wTn
            ups = ps_med.tile([P, DV], F32, tag="psm", name="ups")
            nc.tensor.matmul(ups[:], TTt[:], vb_all[:, c, :], start=True, stop=True)
            u0 = chk.tile([P, DV], F16, tag="u0", bufs=3, name="u0")
            nc.scalar.copy(u0[:], ups[:])
            res["u0"] = u0
            st3[c] = res

        def S4(c):
            pr = st3.pop(c)
            Sold, Snew = S16[(c + 1) % 2], S16[c % 2]
            if c == 0:
                u16 = pr["u0"]
            else:
                ups = ps_med.tile([P, DV], F32, tag="psm", name="ups_s")
                for kt in range(2):
                    nc.tensor.matmul(ups[:], pr["wTn"][:, kt, :], Sold[:, kt, :],
                                     start=(kt == 0), stop=(kt == 1))
                u16 = chk.tile([P, DV], F16, tag="u16", bufs=2, name="u16")
                nc.vector.tensor_tensor(u16[:], ups[:], pr["u0"][:], op=ALU.add)
            # S update first: it is the cross-chunk critical path
            # (S(c-1) -> ups -> u16 -> dps -> S(c)); o can fill in after
            dps = ps_big.tile([P, 2, DV], F32, tag="psb", name="dps")
            for kt in range(2):
                nc.tensor.matmul(dps[:, kt, :], pr["khat"][:, kt * P:(kt + 1) * P], u16[:],
                                 start=True, stop=True)
            if c == 0:
                nc.vector.tensor_copy(Snew[:], dps[:])
            else:
                nc.vector.tensor_tensor(Snew[:], Sold[:], dps[:], op=ALU.add)
            ops = ps_med.tile([P, DV], F32, tag="psm", name="ops")
            if c == 0:
                nc.tensor.matmul(ops[:], pr["attnT"][:], u16[:], start=True, stop=True)
            else:
                for kt in range(2):
                    nc.tensor.matmul(ops[:], pr["qTsl"][kt], Sold[:, kt, :],
                                     start=(kt == 0), stop=False)
                nc.tensor.matmul(ops[:], pr["attnT"][:], u16[:], start=False, stop=True)
            nc.scalar.activation(o_all[:, c, :], ops[:], AF.Identity, scale=pr["rinvq"][:])

        # ---- FIR long: per-iteration batches of PE matmuls, dripped via S3 ----
        fir_state = {}
        fch_of = {}   # n -> fch dict

        def fir_mm_thunks(k):
            """Iteration k (0..NCH-1): list of closures, one FIR matmul each."""
            n, b = k // CPN, k % CPN
            pt, half = b // 2, b % 2
            st = fir_state.setdefault(n, {})
            thunks = []
            if half == 0:
                ps = ps_fir.tile([P, NTILE], F32, tag="fir", name="ps_ll")
                st[pt] = ps
                taps = range(0, 16)
            else:
                ps = st[pt]
                taps = range(16, FIR_L)
            for j in taps:
                def mk(j=j, ps=ps, pt=pt, n=n):
                    nc.tensor.matmul(ps[:], fdiag[:, pt, j, :],
                                     vwin[:, pt, n * NTILE + 1 + j: n * NTILE + 1 + j + NTILE],
                                     start=(j == 0), stop=(j == FIR_L - 1))
                thunks.append(mk)
            return thunks

        def fir_finish(k):
            """Evicts + FIR-short chain for iteration k's batch."""
            n, b = k // CPN, k % CPN
            pt, half = b // 2, b % 2
            fch = fch_of.setdefault(n, {})
            if half == 1:
                ll = gat.tile([P, NTILE], F16, tag="llch", bufs=6, name="llch")
                nc.scalar.copy(ll[:], fir_state[n][pt][:])
                fch[("ll", pt)] = ll
            else:
                ls = gat.tile([P, NTILE], F16, tag="lsch", bufs=6, name="lsch")
                bs = FGUARD - FIR_S + 1 + n * NTILE
                nc.vector.tensor_scalar_mul(ls[:], vwin[:, pt, bs:bs + NTILE],
                                            staps[:, pt, 0:1])
                for j in range(1, FIR_S):
                    sprod = gat.tile([P, NTILE], F16, tag="sprod", bufs=2, name="sprod")
                    nc.vector.tensor_scalar_mul(sprod[:], vwin[:, pt, bs + j:bs + j + NTILE],
                                                staps[:, pt, j:j + 1])
                    nc.vector.tensor_tensor(ls[:], ls[:], sprod[:], op=ALU.add)
                fch[("ls", pt)] = ls

        # ================= phase C: gating + output projection =================
        def gate_out(lt, fch):
            off = (lt % CPN) * CH
            cs = lambda t: t[:, lt:lt + 1]
            toks = {}
            for f in ("ls", "ll"):
                tokt = gat.tile([P, DV], F16, tag=f"{f}tok", bufs=2, name=f"{f}tok")
                tp = ps_t.tile([P, 2, CH], F16, tag="pst", name=f"tp_{f}")
                for pt in range(2):
                    nc.tensor.transpose(tp[:, pt, :], fch[(f, pt)][:, off:off + CH], ident[:])
                nc.scalar.copy(tokt[:], tp[:])
                toks[f] = tokt
            # folded gating: 4 DVE tensor_scalar products (2x mode) + 3 Pool adds
            po = gat.tile([P, DV], F16, tag="gtmp", bufs=8, name="po")
            nc.vector.tensor_scalar_mul(po[:], o_all[:, lt, :], cs(wg_c))
            pv = gat.tile([P, DV], F16, tag="gtmp", bufs=8, name="pv")
            nc.vector.tensor_scalar_mul(pv[:], vb_all[:, lt, :], cs(b_c))
            pls = gat.tile([P, DV], F16, tag="gtmp", bufs=8, name="pls")
            nc.vector.tensor_scalar_mul(pls[:], toks["ls"][:], cs(c1_c))
            pll = gat.tile([P, DV], F16, tag="gtmp", bufs=8, name="pll")
            nc.vector.tensor_scalar_mul(pll[:], toks["ll"][:], cs(c2_c))
            s1g = gat.tile([P, DV], F16, tag="gtmp", bufs=8, name="s1g")
            nc.gpsimd.tensor_tensor(s1g[:], po[:], pv[:], op=ALU.add)
            s2g = gat.tile([P, DV], F16, tag="gtmp", bufs=8, name="s2g")
            nc.gpsimd.tensor_tensor(s2g[:], pls[:], pll[:], op=ALU.add)
            om = gat.tile([P, DV], F16, tag="gtmp", bufs=8, name="om")
            nc.gpsimd.tensor_tensor(om[:], s1g[:], s2g[:], op=ALU.add)
            scr = gat.tile([P, DV], F16, tag="scr_g", bufs=2, name="scr_g")
            ssq = gat.tile([P, 1], F32, tag="ssq_g", bufs=2, name="ssq_g")
            nc.vector.scalar_tensor_tensor(scr[:], om[:], 1.0, om[:],
                                           op0=ALU.mult, op1=ALU.mult, accum_out=ssq[:])
            srt = gat.tile([P, 1], F32, tag="srt_g", bufs=2, name="srt_g")
            nc.scalar.activation(srt[:], ssq[:], AF.Sqrt, bias=eps_rms[:], scale=1.0 / DV)
            rin = gat.tile([P, 1], F32, tag="rin_g", bufs=2, name="rin_g")
            nc.vector.reciprocal(rin[:], srt[:])
            # output projection: transpose unnormalized om; fold the RMS scale
            # into the PSUM eviction (per-partition scale on out rows = tokens)
            onT = gat.tile([P, 2, CH], F16, tag="onT", bufs=2, name="onT")
            tp = ps_t.tile([P, 2, CH], F16, tag="pst", name="tp_on")
            for pt in range(2):
                nc.tensor.transpose(tp[:, pt, :], om[:, pt * P:(pt + 1) * P], ident[:])
            nc.scalar.copy(onT[:], tp[:])
            out_sb = gat.tile([P, D], F16, tag="out_sb", bufs=2, name="out_sb")
            for nt2 in range(2):
                ops = ps_big.tile([P, NTILE], F32, tag="psb", name="ops_o")
                for kt in range(2):
                    nc.tensor.matmul(ops[:], onT[:, kt, :],
                                     wo[:, kt, nt2 * NTILE:(nt2 + 1) * NTILE],
                                     start=(kt == 0), stop=(kt == 1))
                nc.scalar.activation(out_sb[:, nt2 * NTILE:(nt2 + 1) * NTILE], ops[:],
                                     AF.Identity, scale=rin[:])
            nc.sync.dma_start(out_d[lt * CH:(lt + 1) * CH, :], out_sb[:])

        # ---- steady-state emission: stages offset by iteration ----
        for it in range(NCH + 4):
            c4, c3, c2, c1 = it - 3, it - 2, it - 1, it
            gl = it - 4
            pend = list(fir_mm_thunks(it)) if it < NCH else []

            def drip(k):
                for _ in range(min(k, len(pend))):
                    pend.pop(0)()

            if 0 <= c4 < NCH:
                S4(c4)
            drip(2)
            if 0 <= c2 < NCH:
                S2(c2, drip)
            if 0 <= c3 < NCH:
                S3(c3, drip)
            drip(len(pend))
            if it < NCH:
                fir_finish(it)
            if 0 <= c1 < NCH:
                S1(c1)
            if 0 <= gl < NCH:
                gate_out(gl, fch_of[gl // CPN])

    nc.compile()
    return nc


# ---------------- host side ----------------

def _diag_stack(taps):
    """taps [C, K] -> [2, K, 128, 128] diag matrices."""
    C, K = taps.shape
    out = np.zeros((2, K, P, P), np.float32)
    for pt in range(2):
        for j in range(K):
            np.fill_diagonal(out[pt, j], taps[pt * P:(pt + 1) * P, j])
    return out


def make_core_inputs(inputs, b, h, L):
    f = lambda a: np.ascontiguousarray(np.asarray(a, np.float32))
    x = f(inputs['hidden_states'])[b]          # [L, D]
    temp = float(np.exp(np.asarray(inputs['log_temp'], np.float64)[h]))
    wsm = np.concatenate([
        f(inputs['Wb'])[:, h:h + 1],
        f(inputs['Wg'])[:, h:h + 1],
        f(inputs['Wl'])[:, 3 * h:3 * h + 3] / temp], axis=1)
    bias5 = np.array([0.0, float(np.asarray(inputs['bg'], np.float64)[h]),
                      *(np.asarray(inputs['bl'], np.float64)[3 * h:3 * h + 3] / temp)],
                     np.float32)
    ct = np.stack([
        f(inputs['conv_q'])[h * DK:(h + 1) * DK].reshape(2, P, CONV_K),
        f(inputs['conv_k'])[h * DK:(h + 1) * DK].reshape(2, P, CONV_K),
        f(inputs['conv_v'])[h * DV:(h + 1) * DV].reshape(2, P, CONV_K)])  # [3, 2, 128, 4]
    # residual FIR taps: fir = delta + r  ->  local = v + FIR_r(v); softmax sums to 1
    fs = f(inputs['fir_short'])[h].copy()   # [DV, 3]
    fs[:, -1] -= 1.0
    fl = f(inputs['fir_long'])[h].copy()    # [DV, 31]
    fl[:, -1] -= 1.0
    fd = _diag_stack(fl).astype(np.float16)
    st = fs.reshape(2, P, FIR_S)
    wo = f(inputs['rms_w'])[:, None] * f(inputs['Wo'])[h * DV:(h + 1) * DV]
    h16 = np.float16
    return dict(
        xT=np.ascontiguousarray(x.T).astype(h16),
        wq=np.ascontiguousarray(f(inputs['Wq'])[:, h * DK:(h + 1) * DK]).astype(h16),
        wk=np.ascontiguousarray(f(inputs['Wk'])[:, h * DK:(h + 1) * DK]).astype(h16),
        wv=np.ascontiguousarray(f(inputs['Wv'])[:, h * DV:(h + 1) * DV]).astype(h16),
        wsm=wsm.astype(h16), bias5=bias5,
        ctaps=ct, fdiag=fd, staps=st.astype(np.float32), wo=wo.astype(h16),
        ident=np.eye(P, dtype=h16),
        masklt=np.tril(np.ones((P, P), h16), -1),
        maskut=np.triu(np.ones((P, P), h16), 0),
    )


_NC_CACHE = {}


def _get_nc(L):
    if L not in _NC_CACHE:
        _NC_CACHE[L] = build(L)
    return _NC_CACHE[L]


def kernel(**inputs):
    x = np.asarray(inputs['hidden_states'])
    Bx, L, _ = x.shape
    nc = _get_nc(L)
    in_maps = [make_core_inputs(inputs, c // H, c % H, L) for c in range(8)]
    res = run_bass_kernel_spmd(nc, in_maps, core_ids=list(range(8)))
    out = np.zeros((Bx, L, D), np.float32)
    for c in range(8):
        out[c // H] += res.results[c]['out'].astype(np.float32)
    return out
